# revision 1
# baseline (speedup 1.0000x reference)
"""Causal multi-head attention (B=4, N=2048, C=1024, H=16) on 8 Trainium2 cores.

Sharding: data-parallel over batch (4) x tensor-parallel over heads (2 groups
of 8).  Core c handles batch c//2, head-group c%2.  Each core computes its
heads' attention and a partial output projection; the host sums the two
head-group partials per batch and adds the bias.

Device layout notes (per core):
  - All matmul operands are bf16; accumulation fp32 in PSUM.
  - x, weights are shipped pre-transposed so QKV lands as q^T/k^T [d, n].
  - Scores are computed transposed (S^T[kv, q]) so softmax's exp feeds the
    PV matmul directly without transposing the probability matrix.
  - No max-subtraction in softmax: scores are O(1) (std ~1) by construction,
    exp never overflows fp32.  The causal mask is added via an
    identity-matmul of an additive mask tile into PSUM before the score
    matmul accumulates on top.
  - The softmax denominator comes for free from a 65th all-ones column
    appended to V (row 64 of the PV PSUM output).
  - Output projection consumes attn^T and produces out^T; the host
    transposes while unsharding.
"""

import numpy as np
import ml_dtypes

BF16 = ml_dtypes.bfloat16

B, N, C, H, D = 4, 2048, 1024, 16, 64
HPC = 8            # heads per core
GD = HPC * D       # 512 channels per head-group
P = 128
KC = C // P        # 8 contraction chunks for the projections
SPAN = 512         # query-column span processed per attention step
NSPAN = N // SPAN
NEG = -28672.0     # additive mask; exactly representable in bf16

_CACHE = {}


def _emit_once(tc, mybir, xT_d, wqkT_d, wvT_d, wpT_d, bm_d, id_d, out_d,
               phases):
    nc = tc.nc
    dt = mybir.dt
    f32, bf = dt.float32, dt.bfloat16
    Exp = mybir.ActivationFunctionType.Exp
    MUL = mybir.AluOpType.mult
    do_qkv = "qkv" in phases
    do_attn = "attn" in phases
    do_proj = "proj" in phases

    with (
        tc.tile_pool(name="weights", bufs=1) as wp,
        tc.tile_pool(name="acts", bufs=1) as ab,
        tc.tile_pool(name="small", bufs=4) as sp,
        tc.tile_pool(name="ps", bufs=1, space="PSUM") as ps,
        tc.tile_pool(name="aTp", bufs=2) as aTp,
        tc.tile_pool(name="exp", bufs=4) as exp_pool,
    ):
        # ---------------- input loads (chunked: DMA parallelism + fine deps)
        xk = [[wp.tile([P, N // 2], bf, tag=f"xk{k}_{h2}", name=f"xk{k}_{h2}")
               for h2 in range(2)] for k in range(KC)]
        wqk = [wp.tile([P, 2 * GD], bf, tag=f"wqk{k}", name=f"wqk{k}")
               for k in range(KC)]
        wv = [wp.tile([P, GD], bf, tag=f"wv{k}", name=f"wv{k}")
              for k in range(KC)]
        for k in range(KC):
            for h2 in range(2):
                nc.sync.dma_start(
                    xk[k][h2],
                    xT_d[k * P:(k + 1) * P,
                         h2 * (N // 2):(h2 + 1) * (N // 2)])
            nc.sync.dma_start(wqk[k], wqkT_d[k * P:(k + 1) * P, :])
            nc.sync.dma_start(wv[k], wvT_d[k * P:(k + 1) * P, :])
        wpk = [wp.tile([P, C], bf, tag=f"wpk{k}", name=f"wpk{k}")
               for k in range(GD // P)]
        for k in range(GD // P):
            nc.sync.dma_start(wpk[k], wpT_d[k * P:(k + 1) * P, :])
        bm = wp.tile([P, 2 * SPAN], bf, tag="bm")
        nc.sync.dma_start(bm, bm_d)
        i128 = wp.tile([P, P], bf, tag="i128")
        nc.sync.dma_start(i128, id_d)

        # q^T/k^T rows: per (128-row chunk, 512-col quarter) tiles so
        # attention can start before a chunk's later columns are computed
        qkm = [[ab.tile([P, SPAN], bf, tag=f"qkm{m}_{q}", name=f"qkm{m}_{q}")
                for q in range(4)] for m in range(2 * GD // P)]
        # V per kv-block with an all-ones 65th column per head
        vab = [ab.tile([P, HPC * (D + 1)], bf, tag=f"vab{m}", name=f"vab{m}")
               for m in range(N // P)]

        # PSUM bank budget (8 banks of [128, 512]f32):
        #   qk (QKV groups)     [128, 2, 512] x1  = 2
        #   duo/pp              [128, 2, 512] x2  = 4
        #   oA, oB              [65, 512]     x1  = 2
        def qk_chunk(m):
            if not do_qkv:
                return
            for q in range(4):
                pg = ps.tile([P, SPAN], f32, tag="qk", name=f"pg{m}{q}",
                             bufs=2)
                for k in range(KC):
                    nc.tensor.matmul(
                        pg,
                        wqk[k][:, m * P:(m + 1) * P],
                        xk[k][q // 2][:, (q % 2) * SPAN:(q % 2 + 1) * SPAN],
                        start=(k == 0),
                        stop=(k == KC - 1),
                    )
                nc.vector.tensor_copy(out=qkm[m][q], in_=pg)

        def v_chunk(m4):
            if not do_qkv:
                return
            for sub in range(4):
                m16 = m4 * 4 + sub
                pv = ps.tile([P, SPAN], f32, tag="qk", name=f"pv{m16}",
                             bufs=2)
                nc.vector.memset(vab[m16], 1.0)
                for k in range(KC):
                    nc.tensor.matmul(
                        pv,
                        xk[k][m16 // 8][:, (m16 % 8) * P:(m16 % 8 + 1) * P],
                        wv[k],
                        start=(k == 0),
                        stop=(k == KC - 1),
                    )
                nc.vector.tensor_copy(
                    out=vab[m16].rearrange(
                        "p (h e) -> p h e", h=HPC)[:, :, :D],
                    in_=pv.rearrange("p (h e) -> p h e", h=HPC),
                )

        def attn_pair(J, hp, acT):
            if not do_attn:
                return
            nblk = 4 * (J + 1)
            qs = J * SPAN
            outs = (
                ps.tile([65, SPAN], f32, tag="oA", name="oA", bufs=1),
                ps.tile([65, SPAN], f32, tag="oB", name="oB", bufs=1),
            )
            def emit_pv(ex, j2, lo):
                for hi in (0, 1):
                    h = 2 * hp + hi
                    nc.tensor.matmul(
                        outs[hi][:, lo:],
                        vab[j2][:, h * (D + 1):(h + 1) * (D + 1)],
                        ex[:, hi, lo:],
                        start=(j2 == 0),
                        stop=(j2 == nblk - 1),
                    )

            pend = None  # software pipeline: PV one block behind scores/exp
            for j2 in range(nblk):
                duo = ps.tile([P, 2, SPAN], f32, tag="duo", bufs=2)
                dtg = j2 - 4 * J   # >=0: diagonal block index
                lo = P * dtg if dtg >= 0 else 0  # first live column
                diag = dtg >= 0
                if diag:
                    # triangle masks for both heads first, so the two score
                    # matmuls issue back-to-back and row-pack concurrently
                    for hi in (0, 1):
                        nc.tensor.matmul(
                            duo[:, hi, lo:lo + P], i128,
                            bm[:, SPAN:SPAN + P],
                            start=True, stop=False,
                        )
                for hi in (0, 1):
                    nc.tensor.matmul(
                        duo[:, hi, lo:],
                        qkm[4 + hp][j2 // 4][64 * hi:64 * (hi + 1),
                                             (j2 % 4) * P:(j2 % 4 + 1) * P],
                        qkm[hp][J][64 * hi:64 * (hi + 1), lo:],
                        start=not diag,
                        stop=True,
                    )
                ex = exp_pool.tile([P, 2, SPAN], bf, tag="ex")
                nc.scalar.activation(ex[:, :, lo:], duo[:, :, lo:], Exp)
                if pend is not None:
                    emit_pv(*pend)
                pend = (ex, j2, lo)
            emit_pv(*pend)
            for hi in (0, 1):
                o = outs[hi]
                rc = sp.tile([1, SPAN], f32, tag="rc")
                nc.vector.reciprocal(rc, o[64:65, :])
                bc = sp.tile([64, SPAN], f32, tag="bc")
                nc.gpsimd.partition_broadcast(bc, rc)
                nc.vector.tensor_tensor(
                    acT[64 * hi:64 * (hi + 1), hp, :], o[0:64, :], bc, MUL,
                )

        def proj_span(J, acT):
            if not do_proj:
                return
            qs = J * SPAN
            for mo in range(C // P):
                pp = ps.tile([P, SPAN], f32, tag="duo", name=f"pp{mo}",
                             bufs=2)
                for k in range(GD // P):
                    nc.tensor.matmul(
                        pp,
                        wpk[k][:, mo * P:(mo + 1) * P],
                        acT[:, k, :],
                        start=(k == 0),
                        stop=(k == GD // P - 1),
                    )
                ob = sp.tile([P, SPAN], f32, tag="ob")
                nc.vector.tensor_copy(out=ob, in_=pp)
                nc.sync.dma_start(out_d[mo * P:(mo + 1) * P, qs:qs + SPAN],
                                  ob)

        # Interleaved emission: attention (span J, pair hp) needs qkm[hp],
        # qkm[4+hp], vab[0..4J+3]; unblock hp pairs of span 0 early so ACT
        # overlaps the QKV phase.
        acTs = [aTp.tile([P, GD // P, SPAN], bf, tag="acT", name=f"acT{J}")
                for J in range(NSPAN)]
        qk_chunk(0)
        qk_chunk(4)
        v_chunk(0)
        attn_pair(0, 0, acTs[0])
        qk_chunk(1)
        qk_chunk(5)
        attn_pair(0, 1, acTs[0])
        qk_chunk(2)
        qk_chunk(6)
        attn_pair(0, 2, acTs[0])
        qk_chunk(3)
        qk_chunk(7)
        attn_pair(0, 3, acTs[0])
        v_chunk(1)
        attn_pair(1, 0, acTs[1])
        proj_span(0, acTs[0])
        for hp in range(1, 4):
            attn_pair(1, hp, acTs[1])
        v_chunk(2)
        attn_pair(2, 0, acTs[2])
        proj_span(1, acTs[1])
        for hp in range(1, 4):
            attn_pair(2, hp, acTs[2])
        v_chunk(3)
        attn_pair(3, 0, acTs[3])
        proj_span(2, acTs[2])
        for hp in range(1, 4):
            attn_pair(3, hp, acTs[3])
        proj_span(3, acTs[3])


def _emit(tc, mybir, reps=1, phases=("qkv", "attn", "proj")):
    nc = tc.nc
    dt = mybir.dt
    f32, bf = dt.float32, dt.bfloat16

    xT_d = nc.dram_tensor("xT", [C, N], bf, kind="ExternalInput").ap()
    wqkT_d = nc.dram_tensor("wqkT", [C, 2 * GD], bf, kind="ExternalInput").ap()
    wvT_d = nc.dram_tensor("wvT", [C, GD], bf, kind="ExternalInput").ap()
    wpT_d = nc.dram_tensor("wpT", [GD, C], bf, kind="ExternalInput").ap()
    bm_d = nc.dram_tensor("BM", [P, 2 * SPAN], bf, kind="ExternalInput").ap()
    id_d = nc.dram_tensor("I128", [P, P], bf, kind="ExternalInput").ap()
    out_d = nc.dram_tensor("outT", [C, N], f32, kind="ExternalOutput").ap()

    for _rep in range(reps):
        _emit_once(tc, mybir, xT_d, wqkT_d, wvT_d, wpT_d, bm_d, id_d, out_d,
                   phases)


def _get_module(reps=1, phases=("qkv", "attn", "proj")):
    key = (reps, tuple(phases))
    if key not in _CACHE:
        import concourse.tile as tile
        from concourse import bacc, mybir

        nc = bacc.Bacc("TRN2", target_bir_lowering=False, debug=False,
                       num_devices=8)
        with tile.TileContext(nc) as tc:
            _emit(tc, mybir, reps=reps, phases=phases)
        nc.compile()
        _CACHE[key] = nc
    return _CACHE[key]


def _host_inputs(x, w_qkv, w_proj):
    scale = D ** -0.5
    bmask = np.full((P, 2 * SPAN), NEG, np.float32)
    for p in range(P):
        bmask[p, p + SPAN:] = 0.0
    bmask = bmask.astype(BF16)
    ident = np.eye(P, dtype=BF16)
    in_maps = []
    for core in range(8):
        b, g = core // 2, core % 2
        rows = slice(g * GD, (g + 1) * GD)
        wq = w_qkv[0 * C:1 * C][rows] * scale
        wk = w_qkv[1 * C:2 * C][rows]
        wv = w_qkv[2 * C:3 * C][rows]
        in_maps.append({
            "xT": np.ascontiguousarray(x[b].T).astype(BF16),
            "wqkT": np.ascontiguousarray(
                np.concatenate([wq, wk], axis=0).T).astype(BF16),
            "wvT": np.ascontiguousarray(wv.T).astype(BF16),
            "wpT": np.ascontiguousarray(w_proj[:, rows].T).astype(BF16),
            "BM": bmask,
            "I128": ident,
        })
    return in_maps


def kernel(x, w_qkv, w_proj, b_proj, _trace=False):
    from concourse.bass_utils import run_bass_kernel_spmd

    nc = _get_module()
    in_maps = _host_inputs(np.asarray(x, np.float32),
                           np.asarray(w_qkv, np.float32),
                           np.asarray(w_proj, np.float32))
    res = run_bass_kernel_spmd(nc, in_maps, core_ids=list(range(8)),
                               trace=_trace)
    outs = [r["outT"] for r in res.results]
    out = np.empty((B, N, C), np.float32)
    bp = np.asarray(b_proj, np.float32)[None, :]
    for b in range(B):
        out[b] = outs[2 * b].T + outs[2 * b + 1].T + bp
    if _trace:
        kernel._last_results = res
    return out



# revision 18
# speedup vs baseline: 1.4163x; 1.4163x over previous
"""Causal multi-head attention (B=4, N=2048, C=1024, H=16) on 8 Trainium2 cores.

Sharding: data-parallel over batch (4) x tensor-parallel over heads (2 groups
of 8).  Core c handles batch c//2, head-group c%2.  Each core computes its
heads' attention and a partial output projection; the host sums the two
head-group partials per batch and adds the bias.

Device layout notes (per core):
  - All matmul operands are bf16; accumulation fp32 in PSUM.
  - x, weights shipped pre-transposed; qkv projection produces q^T/k^T [d, n]
    and V [n, d+1] (65th all-ones column gives softmax denominators for free).
  - Scores are computed transposed (S^T[kv, q]) so softmax's exp output feeds
    the PV matmuls as the *stationary* operand.
  - PV is computed output-transposed: out[q, d+1] per (q-subtile, head) with
    the probability block as stationary and V as moving -- the streamed free
    dimension is only 65 instead of up-to-512 query columns.
  - No max-subtraction in softmax: scores are O(1) by construction.  Causal
    mask added via an identity-matmul of an additive mask tile into PSUM.
  - Normalization: per-partition reciprocal of the denominator column times
    the PV output (DVE tensor_scalar), written as [q, c] bf16; a DMA-XBAR
    transpose instruction produces [c, q] for the output projection.
  - The output projection emits out^T in bf16; host transposes/sums/biases.
  - Emission interleaves QKV / V / projection work into the attention block
    stream ("filler") so the tensor engine never idles while the activation
    engine works through the exps.
"""

import numpy as np
import ml_dtypes
from collections import deque

BF16 = ml_dtypes.bfloat16

B, N, C, H, D = 4, 2048, 1024, 16, 64
HPC = 8            # heads per core
GD = HPC * D       # 512 channels per head-group
P = 128
KC = C // P        # 8 contraction chunks for the projections
SPAN = 512         # query-column span per attention span
NSPAN = N // SPAN
DP1 = D + 1        # head dim + denominator column
NEG = -28672.0     # additive mask; exactly representable in bf16
NWARM = 72         # PE p-state warmup matmuls
QUOTA_J = [1900, 1550, 1250, 1020]   # filler rows pulled per block

_CACHE = {}
_PHASE = ["init"]     # emission-phase label hook for trace attribution


def _lbl(s):
    _PHASE[0] = s


class _Filler:
    """FIFO of named generators, each yielding the PE rows just emitted."""

    def __init__(self):
        self.q = deque()
        self.done = set()
        self.debt = 0.0

    def add(self, name, gen):
        self.q.append((name, gen))

    def _step(self):
        name, g = self.q[0]
        r = next(g, None)
        if r is None:
            self.done.add(name)
            self.q.popleft()
            return 0
        return r

    def pull(self, rows):
        self.debt += rows
        while self.debt > 0 and self.q:
            self.debt -= self._step()

    def ensure(self, names):
        names = [n for n in names if n not in self.done]
        while names:
            self._step()
            names = [n for n in names if n not in self.done]

    def run_all(self):
        while self.q:
            self._step()


def _emit_once(tc, mybir, xT_d, wall_d, wpT_d, misc_d, out_d):
    nc = tc.nc
    dt = mybir.dt
    f32, bf = dt.float32, dt.bfloat16
    Exp = mybir.ActivationFunctionType.Exp
    MUL = mybir.AluOpType.mult

    xr = xT_d.rearrange("(k p) n -> p k n", p=P)       # [128, 8, 2048]
    wr = wall_d.rearrange("(k p) o -> p k o", p=P)     # [128, 8, 1536]
    pr = wpT_d.rearrange("(k p) o -> p k o", p=P)      # [128, 4, 1024]
    outr = out_d.rearrange("(m p) n -> p m n", p=P)    # [128, 8, 2048]

    with (
        tc.tile_pool(name="weights", bufs=1) as wp,
        tc.tile_pool(name="acts", bufs=1) as ab,
        tc.tile_pool(name="small", bufs=4) as sp,
        tc.tile_pool(name="ps", bufs=1, space="PSUM") as ps,
        tc.tile_pool(name="aTp", bufs=2) as aTp,
        tc.tile_pool(name="exp", bufs=4) as exp_pool,
    ):
        # ---------------- persistent tiles
        wall = wp.tile([P, KC, 3 * GD], bf, tag="wall", name="wall")
        xq = [wp.tile([P, KC, SPAN], bf, tag=f"xq{q}", name=f"xq{q}")
              for q in range(4)]
        wpk = wp.tile([P, GD // P, C], bf, tag="wpk", name="wpk")
        misc = wp.tile([P, 3 * P], bf, tag="misc", name="misc")
        i128 = misc[:, :P]       # identity (PE transpose)
        # 0/1 causal keep-mask, duplicated for the two heads of a pair
        tri2 = misc[:, P:].rearrange("p (h f) -> p h f", h=2)
        warm = wp.tile([P, 64], bf, tag="warm", name="warm")

        qkm = [[ab.tile([P, SPAN], bf, tag=f"qkm{m}_{q}", name=f"qkm{m}_{q}")
                for q in range(4)] for m in range(2 * GD // P)]
        vab = [ab.tile([P, HPC * DP1], bf, tag=f"vab{m}", name=f"vab{m}")
               for m in range(N // P)]

        # ---------------- PE warmup (p-state ramp) while DMAs land
        _lbl("warmup")
        nc.vector.memset(warm, 0.0)
        wps = ps.tile([P, SPAN], f32, tag="qk", bufs=2, name="wps")
        for _ in range(NWARM):
            nc.tensor.matmul(wps[:64, :64], warm[:, :64], warm[:, :64],
                             start=True, stop=True)

        # ---------------- input DMAs (SP queue), latency-ordered
        # wall columns are host-reordered [m0, m4, v, m1, m5, m2, m6, m3, m7]
        # so the first-needed stationaries + v weights land first.
        _lbl("dma_in")
        for kk in range(4):
            k2 = slice(2 * kk, 2 * kk + 2)
            nc.sync.dma_start(wall[:, k2, :768], wr[:, k2, :768])
            nc.sync.dma_start(xq[0][:, k2, :], xr[:, k2, 0:SPAN])
        nc.sync.dma_start(misc, misc_d)
        for kk in range(4):
            k2 = slice(2 * kk, 2 * kk + 2)
            nc.sync.dma_start(wall[:, k2, 768:], wr[:, k2, 768:])
            nc.sync.dma_start(xq[1][:, k2, :], xr[:, k2, SPAN:2 * SPAN])
        nc.sync.dma_start(xq[2], xr[:, :, 2 * SPAN:3 * SPAN])
        nc.sync.dma_start(xq[3], xr[:, :, 3 * SPAN:4 * SPAN])
        nc.sync.dma_start(wpk, pr)

        # ---------------- work-unit generators
        # PSUM bank budget (8 banks of [128, 2KB]):
        #   duo  [128, 2, 512]f32 x2   = 4 banks (score pipeline)
        #   qk   [128, 512]f32    x2   = 2 banks (qkv/v/proj filler units)
        #   pvt  [128, 4, 128]f32 x2   = 2 banks (PV accumulators, 1/head)
        WCOL = {0: 0, 4: 128, 1: 768, 5: 896, 2: 1024, 6: 1152,
                3: 1280, 7: 1408}

        def qk_unit(m, q, tag="qk", tbufs=2):
            _lbl(f"qk_unit")
            pg = ps.tile([P, SPAN], f32, tag=tag, bufs=tbufs,
                         name=f"pg{m}{q}")
            for k in range(KC):
                _lbl("qk_unit")
                nc.tensor.matmul(pg, wall[:, k, WCOL[m]:WCOL[m] + P],
                                 xq[q][:, k, :],
                                 start=(k == 0), stop=(k == KC - 1))
                yield SPAN
            nc.vector.tensor_copy(out=qkm[m][q], in_=pg)
            yield 0

        def v_unit(b16, tag="qk", tbufs=2):
            _lbl(f"v_unit")
            nc.gpsimd.memset(vab[b16], 1.0)
            pv = ps.tile([P, SPAN], f32, tag=tag, bufs=tbufs,
                         name=f"pv{b16}")
            qb, cb = divmod(b16, 4)
            for k in range(KC):
                _lbl("v_unit")
                nc.tensor.matmul(pv, xq[qb][:, k, cb * P:(cb + 1) * P],
                                 wall[:, k, 256:768],
                                 start=(k == 0), stop=(k == KC - 1))
                yield SPAN
            nc.vector.tensor_copy(
                out=vab[b16].rearrange("p (h e) -> p h e", h=HPC)[:, :, :D],
                in_=pv.rearrange("p (h e) -> p h e", h=HPC),
            )
            yield 0

        obslot = [None]

        def proj_unit(J, mo, acTJ, tag="qk", tbufs=2):
            _lbl(f"proj{J}")
            pp = ps.tile([P, SPAN], f32, tag=tag, name=f"pp{J}{mo}",
                         bufs=tbufs)
            for k in range(GD // P):
                _lbl(f"proj{J}")
                nc.tensor.matmul(pp, wpk[:, k, mo * P:(mo + 1) * P],
                                 acTJ[:, k, :],
                                 start=(k == 0), stop=(k == GD // P - 1))
                yield SPAN
            if mo % 2 == 0:
                obslot[0] = sp.tile([P, 2, SPAN], bf, tag="ob", bufs=4,
                                    name="ob")
            nc.vector.tensor_copy(out=obslot[0][:, mo % 2, :], in_=pp)
            if J == NSPAN - 1:
                # tail: per-mo output DMAs shorten the final drain
                nc.sync.dma_start(
                    outr[:, mo:mo + 1, J * SPAN:(J + 1) * SPAN],
                    obslot[0][:, mo % 2:mo % 2 + 1, :])
            elif mo % 2 == 1:
                nc.sync.dma_start(
                    outr[:, mo - 1:mo + 1, J * SPAN:(J + 1) * SPAN],
                    obslot[0])
            yield 0

        # ---------------- attention
        def attn_pair(J, hp, asmP, filler):
            nblk = 4 * (J + 1)
            pvt = [ps.tile([P, 4, P], f32, tag="pvt", bufs=2,
                           name=f"pvt{J}{hp}{hi}")
                   for hi in (0, 1)]
            # start_tensor_calc zeroes the whole 2KB PSUM bank, so the four
            # per-subtile accumulation groups sharing one bank must NOT each
            # use start=True.  One dummy F=1 matmul opens + zeroes the bank;
            # every PV matmul then accumulates (start=False), and the final
            # write (s=3 diagonal) closes the group.
            for hi in (0, 1):
                nc.tensor.matmul(pvt[hi][:, 0, P - 1:P], i128, warm[:, 0:1],
                                 start=True, stop=False)

            def emit_pv(ex, j2, dtg):
                _lbl(f"pv_J{J}")
                for s in range(4):
                    if s < dtg:
                        continue
                    for hi in (0, 1):
                        h = 2 * hp + hi
                        nc.tensor.matmul(
                            pvt[hi][:, s, :DP1],
                            ex[:, hi, s * P:(s + 1) * P],
                            vab[j2][:, h * DP1:(h + 1) * DP1],
                            start=False,
                            stop=(s == 3 and j2 == 4 * J + 3),
                        )

            pend = deque()
            for j2 in range(nblk):
                _lbl(f"scores_J{J}")
                duo = ps.tile([P, 2, SPAN], f32, tag="duo", bufs=2, name="duo")
                dtg = j2 - 4 * J
                diag = dtg >= 0
                lo = P * dtg if diag else 0
                for hi in (0, 1):
                    kst = qkm[4 + hp][j2 // 4][64 * hi:64 * (hi + 1),
                                               (j2 % 4) * P:(j2 % 4 + 1) * P]
                    qmv = qkm[hp][J][64 * hi:64 * (hi + 1), :]
                    nc.tensor.matmul(duo[:, hi, lo:], kst, qmv[:, lo:],
                                     start=True, stop=True)
                _lbl(f"exp_J{J}")
                ex = exp_pool.tile([P, 2, SPAN], bf, tag="ex", bufs=4, name="ex")
                nc.scalar.activation(ex[:, :, lo:], duo[:, :, lo:], Exp)
                if diag:
                    # zero the dead (q < kv) triangle of the diagonal block
                    nc.vector.tensor_tensor(ex[:, :, lo:lo + P],
                                            ex[:, :, lo:lo + P], tri2, MUL)
                pend.append((ex, j2, dtg))
                if len(pend) > 2:
                    emit_pv(*pend.popleft())
                filler.pull(QUOTA_J[J])
            while pend:
                emit_pv(*pend.popleft())
            _lbl(f"norm_J{J}")
            rcb = [sp.tile([P, 4, 1], f32, tag=f"rc{hi}", bufs=2,
                           name=f"rcb{hi}") for hi in (0, 1)]
            for hi in (0, 1):
                nc.vector.reciprocal(rcb[hi], pvt[hi][:, :, D:DP1])
            for s in range(4):
                for hi in (0, 1):
                    nc.vector.tensor_scalar(
                        asmP[:, s, hi * D:(hi + 1) * D],
                        pvt[hi][:, s, :D], rcb[hi][:, s, :], None, MUL)

        # ---------------- schedule
        def drain_rr(gens):
            gens = list(gens)
            while gens:
                gens = [g for g in gens if next(g, None) is not None]

        # Phase A: first qk chunks + V blocks 0..3 (paced by input DMAs).
        # Borrow the idle duo ring so three units can be in flight at once.
        drain_rr([qk_unit(0, 0), qk_unit(4, 0, "duo", 2), v_unit(0, "duo", 2)])
        drain_rr([v_unit(1), v_unit(2, "duo", 2), v_unit(3, "duo", 2)])

        filler = _Filler()
        order = []
        for J in range(4):
            for pair in (1, 2, 3):
                order += [(f"qk{pair}_{J}", qk_unit(pair, J)),
                          (f"qk{4 + pair}_{J}", qk_unit(4 + pair, J))]
            if J < 3:
                order += [(f"qk0_{J + 1}", qk_unit(0, J + 1)),
                          (f"qk4_{J + 1}", qk_unit(4, J + 1))]
                order += [(f"v{b}", v_unit(b))
                          for b in range(4 * (J + 1), 4 * (J + 1) + 4)]
        for name, gen in order:
            filler.add(name, gen)

        for J in range(NSPAN):
            acTJ = aTp.tile([P, GD // P, SPAN], bf, tag="acT", bufs=3,
                            name=f"acT{J}")
            for hp in range(4):
                need = [f"qk{hp}_{J}", f"qk{4 + hp}_{J}"]
                if hp == 0 and J > 0:
                    need += [f"v{b}" for b in range(4 * J, 4 * J + 4)]
                filler.ensure([n for n in need
                               if any(n == nm for nm, _ in order)])
                asmP = aTp.tile([P, 4, P], bf, tag=f"asmp{hp}", bufs=2,
                                name=f"asmp{J}_{hp}")
                attn_pair(J, hp, asmP, filler)
                _lbl(f"transpose_J{J}")
                nc.sync.dma_start(
                    acTJ[:, hp, :].rearrange("p (s q) -> p s q", s=4),
                    asmP, transpose=True)
            if J < NSPAN - 1:
                for mo in range(C // P):
                    filler.add(f"proj{J}_{mo}", proj_unit(J, mo, acTJ))
            else:
                for mo in range(C // P):
                    filler.add(f"proj{J}_{mo}", proj_unit(J, mo, acTJ))
        filler.run_all()


def _emit(tc, mybir, reps=1, phases=None):
    nc = tc.nc
    dt = mybir.dt
    f32, bf = dt.float32, dt.bfloat16

    xT_d = nc.dram_tensor("xT", [C, N], bf, kind="ExternalInput").ap()
    wall_d = nc.dram_tensor("WALL", [C, 3 * GD], bf,
                            kind="ExternalInput").ap()
    wpT_d = nc.dram_tensor("wpT", [GD, C], bf, kind="ExternalInput").ap()
    misc_d = nc.dram_tensor("MISC", [P, 3 * P], bf,
                            kind="ExternalInput").ap()
    out_d = nc.dram_tensor("outT", [C, N], bf, kind="ExternalOutput").ap()

    for _rep in range(reps):
        _emit_once(tc, mybir, xT_d, wall_d, wpT_d, misc_d, out_d)


def _get_module(reps=1, phases=None):
    key = (reps,)
    if key not in _CACHE:
        import concourse.tile as tile
        from concourse import bacc, mybir

        nc = bacc.Bacc("TRN2", target_bir_lowering=False, debug=False,
                       num_devices=8)
        with tile.TileContext(nc) as tc:
            _emit(tc, mybir, reps=reps)
        nc.compile()
        _CACHE[key] = nc
    return _CACHE[key]


def _host_inputs(x, w_qkv, w_proj):
    scale = D ** -0.5
    tri01 = np.zeros((P, P), np.float32)
    for p in range(P):
        tri01[p, p:] = 1.0
    misc = np.concatenate([np.eye(P, dtype=np.float32), tri01, tri01],
                          axis=1).astype(BF16)
    in_maps = []
    for core in range(8):
        b, g = core // 2, core % 2
        rows = slice(g * GD, (g + 1) * GD)
        wq = (w_qkv[0 * C:1 * C][rows] * scale).T
        wk = w_qkv[1 * C:2 * C][rows].T
        wv = w_qkv[2 * C:3 * C][rows].T
        # column order [m0, m4, v, m1, m5, m2, m6, m3, m7] (see WCOL)
        wall = np.concatenate(
            [wq[:, 0:128], wk[:, 0:128], wv,
             wq[:, 128:256], wk[:, 128:256],
             wq[:, 256:384], wk[:, 256:384],
             wq[:, 384:512], wk[:, 384:512]], axis=1)  # [C, 1536]
        in_maps.append({
            "xT": np.ascontiguousarray(x[b].T).astype(BF16),
            "WALL": np.ascontiguousarray(wall).astype(BF16),
            "wpT": np.ascontiguousarray(w_proj[:, rows].T).astype(BF16),
            "MISC": misc,
        })
    return in_maps


def kernel(x, w_qkv, w_proj, b_proj, _trace=False):
    from concourse.bass_utils import run_bass_kernel_spmd

    nc = _get_module()
    in_maps = _host_inputs(np.asarray(x, np.float32),
                           np.asarray(w_qkv, np.float32),
                           np.asarray(w_proj, np.float32))
    res = run_bass_kernel_spmd(nc, in_maps, core_ids=list(range(8)),
                               trace=_trace)
    outs = [np.asarray(r["outT"], np.float32) for r in res.results]
    out = np.empty((B, N, C), np.float32)
    bp = np.asarray(b_proj, np.float32)[None, :]
    for b in range(B):
        out[b] = outs[2 * b].T + outs[2 * b + 1].T + bp
    if _trace:
        kernel._last_results = res
    return out


# revision 19
# speedup vs baseline: 1.4457x; 1.0208x over previous
"""Causal multi-head attention (B=4, N=2048, C=1024, H=16) on 8 Trainium2 cores.

Sharding: data-parallel over batch (4) x tensor-parallel over heads (2 groups
of 8).  Core c handles batch c//2, head-group c%2.  Each core computes its
heads' attention and a partial output projection; the host sums the two
head-group partials per batch and adds the bias.

Device layout notes (per core):
  - All matmul operands are bf16; accumulation fp32 in PSUM.
  - x, weights shipped pre-transposed; qkv projection produces q^T/k^T [d, n]
    and V [n, d+1] (65th all-ones column gives softmax denominators for free).
  - Scores are computed transposed (S^T[kv, q]) so softmax's exp output feeds
    the PV matmuls as the *stationary* operand.
  - PV is computed output-transposed: out[q, d+1] per (q-subtile, head) with
    the probability block as stationary and V as moving -- the streamed free
    dimension is only 65 instead of up-to-512 query columns.
  - No max-subtraction in softmax: scores are O(1) by construction.  Causal
    mask added via an identity-matmul of an additive mask tile into PSUM.
  - Normalization: per-partition reciprocal of the denominator column times
    the PV output (DVE tensor_scalar), written as [q, c] bf16; a DMA-XBAR
    transpose instruction produces [c, q] for the output projection.
  - The output projection emits out^T in bf16; host transposes/sums/biases.
  - Emission interleaves QKV / V / projection work into the attention block
    stream ("filler") so the tensor engine never idles while the activation
    engine works through the exps.
"""

import numpy as np
import ml_dtypes
from collections import deque

BF16 = ml_dtypes.bfloat16

B, N, C, H, D = 4, 2048, 1024, 16, 64
HPC = 8            # heads per core
GD = HPC * D       # 512 channels per head-group
P = 128
KC = C // P        # 8 contraction chunks for the projections
SPAN = 512         # query-column span per attention span
NSPAN = N // SPAN
DP1 = D + 1        # head dim + denominator column
NEG = -28672.0     # additive mask; exactly representable in bf16
NWARM = 72         # PE p-state warmup matmuls
QUOTA_J = [1900, 1550, 1250, 1020]   # filler rows pulled per block

_CACHE = {}
_PHASE = ["init"]     # emission-phase label hook for trace attribution


def _lbl(s):
    _PHASE[0] = s


class _Filler:
    """FIFO of named generators, each yielding the PE rows just emitted."""

    def __init__(self):
        self.q = deque()
        self.done = set()
        self.debt = 0.0

    def add(self, name, gen):
        self.q.append((name, gen))

    def _step(self):
        name, g = self.q[0]
        r = next(g, None)
        if r is None:
            self.done.add(name)
            self.q.popleft()
            return 0
        return r

    def pull(self, rows):
        self.debt += rows
        while self.debt > 0 and self.q:
            self.debt -= self._step()

    def ensure(self, names):
        names = [n for n in names if n not in self.done]
        while names:
            self._step()
            names = [n for n in names if n not in self.done]

    def run_all(self):
        while self.q:
            self._step()


def _emit_once(tc, mybir, xT_d, wall_d, wpT_d, misc_d, out_d):
    nc = tc.nc
    dt = mybir.dt
    f32, bf = dt.float32, dt.bfloat16
    Exp = mybir.ActivationFunctionType.Exp
    MUL = mybir.AluOpType.mult

    xr = xT_d.rearrange("(k p) n -> p k n", p=P)       # [128, 8, 2048]
    wr = wall_d.rearrange("(k p) o -> p k o", p=P)     # [128, 8, 1536]
    pr = wpT_d.rearrange("(k p) o -> p k o", p=P)      # [128, 4, 1024]
    outr = out_d.rearrange("(m p) n -> p m n", p=P)    # [128, 8, 2048]

    with (
        tc.tile_pool(name="weights", bufs=1) as wp,
        tc.tile_pool(name="acts", bufs=1) as ab,
        tc.tile_pool(name="small", bufs=4) as sp,
        tc.tile_pool(name="ps", bufs=1, space="PSUM") as ps,
        tc.tile_pool(name="aTp", bufs=2) as aTp,
        tc.tile_pool(name="exp", bufs=4) as exp_pool,
    ):
        # ---------------- persistent tiles
        wall = wp.tile([P, KC, 3 * GD], bf, tag="wall", name="wall")
        xq = [wp.tile([P, KC, SPAN], bf, tag=f"xq{q}", name=f"xq{q}")
              for q in range(4)]
        wpk = wp.tile([P, GD // P, C], bf, tag="wpk", name="wpk")
        misc = wp.tile([P, 3 * P], bf, tag="misc", name="misc")
        i128 = misc[:, :P]       # identity (PE transpose)
        # 0/1 causal keep-mask, duplicated for the two heads of a pair
        tri2 = misc[:, P:].rearrange("p (h f) -> p h f", h=2)
        warm = wp.tile([P, 64], bf, tag="warm", name="warm")

        qkm = [[ab.tile([P, SPAN], bf, tag=f"qkm{m}_{q}", name=f"qkm{m}_{q}")
                for q in range(4)] for m in range(2 * GD // P)]
        vab = [ab.tile([P, HPC * DP1], bf, tag=f"vab{m}", name=f"vab{m}")
               for m in range(N // P)]

        # ---------------- PE warmup (p-state ramp) while DMAs land
        _lbl("warmup")
        nc.vector.memset(warm, 0.0)
        wps = ps.tile([P, SPAN], f32, tag="qk", bufs=2, name="wps")
        for _ in range(NWARM):
            nc.tensor.matmul(wps[:64, :64], warm[:, :64], warm[:, :64],
                             start=True, stop=True)

        # ---------------- input DMAs (SP queue), latency-ordered
        # wall columns are host-reordered [m0, m4, v, m1, m5, m2, m6, m3, m7]
        # so the first-needed stationaries + v weights land first.
        _lbl("dma_in")
        for kk in range(4):
            k2 = slice(2 * kk, 2 * kk + 2)
            nc.sync.dma_start(wall[:, k2, :768], wr[:, k2, :768])
            nc.sync.dma_start(xq[0][:, k2, :], xr[:, k2, 0:SPAN])
        nc.sync.dma_start(misc, misc_d)
        for kk in range(4):
            k2 = slice(2 * kk, 2 * kk + 2)
            nc.sync.dma_start(wall[:, k2, 768:], wr[:, k2, 768:])
            nc.sync.dma_start(xq[1][:, k2, :], xr[:, k2, SPAN:2 * SPAN])
        nc.sync.dma_start(xq[2], xr[:, :, 2 * SPAN:3 * SPAN])
        nc.sync.dma_start(xq[3], xr[:, :, 3 * SPAN:4 * SPAN])
        nc.sync.dma_start(wpk, pr)

        # ---------------- work-unit generators
        # PSUM bank budget (8 banks of [128, 2KB]):
        #   duo  [128, 2, 512]f32 x2   = 4 banks (score pipeline)
        #   qk   [128, 512]f32    x2   = 2 banks (qkv/v/proj filler units)
        #   pvt  [128, 4, 128]f32 x2   = 2 banks (PV accumulators, 1/head)
        WCOL = {0: 0, 4: 128, 1: 768, 5: 896, 2: 1024, 6: 1152,
                3: 1280, 7: 1408}

        def qk_unit(m, q, tag="qk", tbufs=2):
            _lbl(f"qk_unit")
            pg = ps.tile([P, SPAN], f32, tag=tag, bufs=tbufs,
                         name=f"pg{m}{q}")
            for k in range(KC):
                _lbl("qk_unit")
                nc.tensor.matmul(pg, wall[:, k, WCOL[m]:WCOL[m] + P],
                                 xq[q][:, k, :],
                                 start=(k == 0), stop=(k == KC - 1))
                yield SPAN
            nc.vector.tensor_copy(out=qkm[m][q], in_=pg)
            yield 0

        def v_unit(b16, tag="qk", tbufs=2):
            _lbl(f"v_unit")
            nc.gpsimd.memset(vab[b16], 1.0)
            pv = ps.tile([P, SPAN], f32, tag=tag, bufs=tbufs,
                         name=f"pv{b16}")
            qb, cb = divmod(b16, 4)
            for k in range(KC):
                _lbl("v_unit")
                nc.tensor.matmul(pv, xq[qb][:, k, cb * P:(cb + 1) * P],
                                 wall[:, k, 256:768],
                                 start=(k == 0), stop=(k == KC - 1))
                yield SPAN
            nc.vector.tensor_copy(
                out=vab[b16].rearrange("p (h e) -> p h e", h=HPC)[:, :, :D],
                in_=pv.rearrange("p (h e) -> p h e", h=HPC),
            )
            yield 0

        obslot = [None]

        def proj_unit(J, mo, acTc, tag="qk", tbufs=2):
            _lbl(f"proj{J}")
            pp = ps.tile([P, SPAN], f32, tag=tag, name=f"pp{J}{mo}",
                         bufs=tbufs)
            for k in range(GD // P):
                _lbl(f"proj{J}")
                nc.tensor.matmul(pp, wpk[:, k, mo * P:(mo + 1) * P],
                                 acTc[k],
                                 start=(k == 0), stop=(k == GD // P - 1))
                yield SPAN
            if mo % 2 == 0:
                obslot[0] = sp.tile([P, 2, SPAN], bf, tag="ob", bufs=4,
                                    name="ob")
            nc.vector.tensor_copy(out=obslot[0][:, mo % 2, :], in_=pp)
            if J == NSPAN - 1:
                # tail: per-mo output DMAs shorten the final drain
                nc.sync.dma_start(
                    outr[:, mo:mo + 1, J * SPAN:(J + 1) * SPAN],
                    obslot[0][:, mo % 2:mo % 2 + 1, :])
            elif mo % 2 == 1:
                nc.sync.dma_start(
                    outr[:, mo - 1:mo + 1, J * SPAN:(J + 1) * SPAN],
                    obslot[0])
            yield 0

        # ---------------- attention
        def attn_pair(J, hp, asmP, filler):
            nblk = 4 * (J + 1)
            pvt = [ps.tile([P, 4, P], f32, tag="pvt", bufs=2,
                           name=f"pvt{J}{hp}{hi}")
                   for hi in (0, 1)]
            # start_tensor_calc zeroes the whole 2KB PSUM bank, so the four
            # per-subtile accumulation groups sharing one bank must NOT each
            # use start=True.  One dummy F=1 matmul opens + zeroes the bank;
            # every PV matmul then accumulates (start=False), and the final
            # write (s=3 diagonal) closes the group.
            for hi in (0, 1):
                nc.tensor.matmul(pvt[hi][:, 0, P - 1:P], i128, warm[:, 0:1],
                                 start=True, stop=False)

            def emit_pv(ex, j2, dtg):
                _lbl(f"pv_J{J}")
                for s in range(4):
                    if s < dtg:
                        continue
                    for hi in (0, 1):
                        h = 2 * hp + hi
                        nc.tensor.matmul(
                            pvt[hi][:, s, :DP1],
                            ex[:, hi, s * P:(s + 1) * P],
                            vab[j2][:, h * DP1:(h + 1) * DP1],
                            start=False,
                            stop=(s == 3 and j2 == 4 * J + 3),
                        )

            pend = deque()
            for j2 in range(nblk):
                _lbl(f"scores_J{J}")
                duo = ps.tile([P, 2, SPAN], f32, tag="duo", bufs=2, name="duo")
                dtg = j2 - 4 * J
                diag = dtg >= 0
                lo = P * dtg if diag else 0
                for hi in (0, 1):
                    kst = qkm[4 + hp][j2 // 4][64 * hi:64 * (hi + 1),
                                               (j2 % 4) * P:(j2 % 4 + 1) * P]
                    qmv = qkm[hp][J][64 * hi:64 * (hi + 1), :]
                    nc.tensor.matmul(duo[:, hi, lo:], kst, qmv[:, lo:],
                                     start=True, stop=True)
                _lbl(f"exp_J{J}")
                ex = exp_pool.tile([P, 2, SPAN], bf, tag="ex", bufs=4, name="ex")
                nc.scalar.activation(ex[:, :, lo:], duo[:, :, lo:], Exp)
                if diag:
                    # zero the dead (q < kv) triangle of the diagonal block
                    nc.vector.tensor_tensor(ex[:, :, lo:lo + P],
                                            ex[:, :, lo:lo + P], tri2, MUL)
                pend.append((ex, j2, dtg))
                if len(pend) > 2:
                    emit_pv(*pend.popleft())
                filler.pull(QUOTA_J[J])
            while pend:
                emit_pv(*pend.popleft())
            _lbl(f"norm_J{J}")
            rcb = [sp.tile([P, 4, 1], f32, tag=f"rc{hi}", bufs=2,
                           name=f"rcb{hi}") for hi in (0, 1)]
            for hi in (0, 1):
                nc.vector.reciprocal(rcb[hi], pvt[hi][:, :, D:DP1])
            for s in range(4):
                for hi in (0, 1):
                    nc.vector.tensor_scalar(
                        asmP[:, s, hi * D:(hi + 1) * D],
                        pvt[hi][:, s, :D], rcb[hi][:, s, :], None, MUL)

        # ---------------- schedule
        def drain_rr(gens):
            gens = list(gens)
            while gens:
                gens = [g for g in gens if next(g, None) is not None]

        # Phase A: first qk chunks + V blocks 0..3 (paced by input DMAs).
        # Borrow the idle duo ring so three units can be in flight at once.
        drain_rr([qk_unit(0, 0), qk_unit(4, 0, "duo", 2), v_unit(0, "duo", 2)])
        drain_rr([v_unit(1), v_unit(2, "duo", 2), v_unit(3, "duo", 2)])

        filler = _Filler()
        order = []
        for J in range(4):
            for pair in (1, 2, 3):
                order += [(f"qk{pair}_{J}", qk_unit(pair, J)),
                          (f"qk{4 + pair}_{J}", qk_unit(4 + pair, J))]
            if J < 3:
                order += [(f"qk0_{J + 1}", qk_unit(0, J + 1)),
                          (f"qk4_{J + 1}", qk_unit(4, J + 1))]
                order += [(f"v{b}", v_unit(b))
                          for b in range(4 * (J + 1), 4 * (J + 1) + 4)]
        for name, gen in order:
            filler.add(name, gen)

        for J in range(NSPAN):
            acTc = [aTp.tile([P, SPAN], bf, tag=f"acT{k}", bufs=3,
                             name=f"acT{J}_{k}") for k in range(GD // P)]
            for hp in range(4):
                need = [f"qk{hp}_{J}", f"qk{4 + hp}_{J}"]
                if hp == 0 and J > 0:
                    need += [f"v{b}" for b in range(4 * J, 4 * J + 4)]
                filler.ensure([n for n in need
                               if any(n == nm for nm, _ in order)])
                asmP = aTp.tile([P, 4, P], bf, tag=f"asmp{hp}", bufs=2,
                                name=f"asmp{J}_{hp}")
                attn_pair(J, hp, asmP, filler)
                _lbl(f"transpose_J{J}")
                if J == NSPAN - 1 and hp == 3:
                    # The last pair gates the tail; a PE transpose + DVE
                    # copy-out beats the DMA-XBAR round trip by ~1.5us.
                    # Each transpose gets its own PSUM zero region (a duo
                    # ring slot) since start=True zeroes the whole bank.
                    for s in range(4):
                        tps = ps.tile([P, P], bf, tag="duo", bufs=2,
                                      name=f"tps{s}")
                        nc.tensor.transpose(tps, asmP[:, s, :], i128)
                        nc.vector.tensor_copy(
                            out=acTc[hp][:, s * P:(s + 1) * P], in_=tps)
                else:
                    nc.sync.dma_start(
                        acTc[hp].rearrange("p (s q) -> p s q", s=4),
                        asmP, transpose=True)
            for mo in range(C // P):
                filler.add(f"proj{J}_{mo}", proj_unit(J, mo, acTc))
        filler.run_all()


def _emit(tc, mybir, reps=1, phases=None):
    nc = tc.nc
    dt = mybir.dt
    f32, bf = dt.float32, dt.bfloat16

    xT_d = nc.dram_tensor("xT", [C, N], bf, kind="ExternalInput").ap()
    wall_d = nc.dram_tensor("WALL", [C, 3 * GD], bf,
                            kind="ExternalInput").ap()
    wpT_d = nc.dram_tensor("wpT", [GD, C], bf, kind="ExternalInput").ap()
    misc_d = nc.dram_tensor("MISC", [P, 3 * P], bf,
                            kind="ExternalInput").ap()
    out_d = nc.dram_tensor("outT", [C, N], bf, kind="ExternalOutput").ap()

    for _rep in range(reps):
        _emit_once(tc, mybir, xT_d, wall_d, wpT_d, misc_d, out_d)


def _get_module(reps=1, phases=None):
    key = (reps,)
    if key not in _CACHE:
        import concourse.tile as tile
        from concourse import bacc, mybir

        nc = bacc.Bacc("TRN2", target_bir_lowering=False, debug=False,
                       num_devices=8)
        with tile.TileContext(nc) as tc:
            _emit(tc, mybir, reps=reps)
        nc.compile()
        _CACHE[key] = nc
    return _CACHE[key]


def _host_inputs(x, w_qkv, w_proj):
    scale = D ** -0.5
    tri01 = np.zeros((P, P), np.float32)
    for p in range(P):
        tri01[p, p:] = 1.0
    misc = np.concatenate([np.eye(P, dtype=np.float32), tri01, tri01],
                          axis=1).astype(BF16)
    in_maps = []
    for core in range(8):
        b, g = core // 2, core % 2
        rows = slice(g * GD, (g + 1) * GD)
        wq = (w_qkv[0 * C:1 * C][rows] * scale).T
        wk = w_qkv[1 * C:2 * C][rows].T
        wv = w_qkv[2 * C:3 * C][rows].T
        # column order [m0, m4, v, m1, m5, m2, m6, m3, m7] (see WCOL)
        wall = np.concatenate(
            [wq[:, 0:128], wk[:, 0:128], wv,
             wq[:, 128:256], wk[:, 128:256],
             wq[:, 256:384], wk[:, 256:384],
             wq[:, 384:512], wk[:, 384:512]], axis=1)  # [C, 1536]
        in_maps.append({
            "xT": np.ascontiguousarray(x[b].T).astype(BF16),
            "WALL": np.ascontiguousarray(wall).astype(BF16),
            "wpT": np.ascontiguousarray(w_proj[:, rows].T).astype(BF16),
            "MISC": misc,
        })
    return in_maps


def kernel(x, w_qkv, w_proj, b_proj, _trace=False):
    from concourse.bass_utils import run_bass_kernel_spmd

    nc = _get_module()
    in_maps = _host_inputs(np.asarray(x, np.float32),
                           np.asarray(w_qkv, np.float32),
                           np.asarray(w_proj, np.float32))
    res = run_bass_kernel_spmd(nc, in_maps, core_ids=list(range(8)),
                               trace=_trace)
    outs = [np.asarray(r["outT"], np.float32) for r in res.results]
    out = np.empty((B, N, C), np.float32)
    bp = np.asarray(b_proj, np.float32)[None, :]
    for b in range(B):
        out[b] = outs[2 * b].T + outs[2 * b + 1].T + bp
    if _trace:
        kernel._last_results = res
    return out


# revision 41
# speedup vs baseline: 1.4731x; 1.0189x over previous
"""Causal multi-head attention (B=4, N=2048, C=1024, H=16) on 8 Trainium2 cores.

Sharding: data-parallel over batch (4) x tensor-parallel over heads (2 groups
of 8).  Core c handles batch c//2, head-group c%2.  Each core computes its
heads' attention and a partial output projection; the host sums the two
head-group partials per batch and adds the bias.

Device layout notes (per core):
  - All matmul operands are bf16; accumulation fp32 in PSUM.
  - x, weights shipped pre-transposed; qkv projection produces q^T/k^T [d, n]
    and V [n, d+1] (65th all-ones column gives softmax denominators for free).
  - Scores are computed transposed (S^T[kv, q]) so softmax's exp output feeds
    the PV matmuls as the *stationary* operand.
  - PV is computed output-transposed: out[q, d+1] per (q-subtile, head) with
    the probability block as stationary and V as moving -- the streamed free
    dimension is only 65 instead of up-to-512 query columns.
  - No max-subtraction in softmax: scores are O(1) by construction.  The
    causal mask is applied by multiplying the diagonal blocks of the exp
    output with a 0/1 triangle on the vector engine (off the tensor engine).
  - PSUM note: start_tensor_calc zeroes a whole 2KB bank, so the four
    per-subtile PV accumulation groups sharing a bank are opened by one
    dummy matmul and accumulate with start=False.
  - Normalization: per-partition reciprocal of the denominator column times
    the PV output (DVE tensor_scalar), written as [q, c] bf16; a DMA-XBAR
    transpose instruction produces [c, q] for the output projection.
  - The output projection emits out^T in bf16; host transposes/sums/biases.
  - Emission interleaves QKV / V / projection work into the attention block
    stream ("filler") so the tensor engine never idles while the activation
    engine works through the exps.
"""

import numpy as np
import ml_dtypes
from collections import deque

BF16 = ml_dtypes.bfloat16

B, N, C, H, D = 4, 2048, 1024, 16, 64
HPC = 8            # heads per core
GD = HPC * D       # 512 channels per head-group
P = 128
KC = C // P        # 8 contraction chunks for the projections
SPAN = 512         # query-column span per attention span
NSPAN = N // SPAN
DP1 = D + 1        # head dim + denominator column
NWARM = 84         # PE p-state warmup matmuls
QUOTA_J = [2480, 1650, 1010, 1030]   # filler rows pulled per block

_CACHE = {}
_PHASE = ["init"]     # emission-phase label hook for trace attribution


def _lbl(s):
    _PHASE[0] = s


class _Filler:
    """FIFO of named generators, each yielding the PE rows just emitted."""

    def __init__(self):
        self.q = deque()
        self.done = set()
        self.debt = 0.0

    def add(self, name, gen):
        self.q.append((name, gen))

    def _step(self):
        name, g = self.q[0]
        r = next(g, None)
        if r is None:
            self.done.add(name)
            self.q.popleft()
            return 0
        return r

    def pull(self, rows):
        self.debt += rows
        while self.debt > 0 and self.q:
            self.debt -= self._step()

    def ensure(self, names):
        names = [n for n in names if n not in self.done]
        while names:
            self._step()
            names = [n for n in names if n not in self.done]

    def run_all(self):
        while self.q:
            self._step()


def _emit_once(tc, mybir, xT_d, wall_d, wpT_d, misc_d, out_d):
    nc = tc.nc
    dt = mybir.dt
    f32, bf = dt.float32, dt.bfloat16
    Exp = mybir.ActivationFunctionType.Exp
    MUL = mybir.AluOpType.mult

    xr = xT_d.rearrange("(k p) n -> p k n", p=P)       # [128, 8, 2048]
    wr = wall_d.rearrange("(k p) o -> p k o", p=P)     # [128, 8, 1536]
    pr = wpT_d.rearrange("(k p) o -> p k o", p=P)      # [128, 4, 1024]
    outr = out_d.rearrange("(m p) n -> p m n", p=P)    # [128, 8, 2048]

    with (
        tc.tile_pool(name="weights", bufs=1) as wp,
        tc.tile_pool(name="acts", bufs=1) as ab,
        tc.tile_pool(name="small", bufs=4) as sp,
        tc.tile_pool(name="ps", bufs=1, space="PSUM") as ps,
        tc.tile_pool(name="aTp", bufs=2) as aTp,
        tc.tile_pool(name="exp", bufs=4) as exp_pool,
    ):
        # ---------------- persistent tiles
        wall = wp.tile([P, KC, 3 * GD], bf, tag="wall", name="wall")
        xq = [wp.tile([P, KC, SPAN], bf, tag=f"xq{q}", name=f"xq{q}")
              for q in range(4)]
        wpk = wp.tile([P, GD // P, C], bf, tag="wpk", name="wpk")
        misc = wp.tile([P, 3 * P], bf, tag="misc", name="misc")
        i128 = misc[:, :P]       # identity (PE transpose)
        # 0/1 causal keep-mask, duplicated for the two heads of a pair
        tri2 = misc[:, P:].rearrange("p (h f) -> p h f", h=2)
        warm = wp.tile([P, 64], bf, tag="warm", name="warm")

        qkm = [[ab.tile([P, SPAN], bf, tag=f"qkm{m}_{q}", name=f"qkm{m}_{q}")
                for q in range(4)] for m in range(2 * GD // P)]
        vab = [ab.tile([P, HPC * DP1], bf, tag=f"vab{m}", name=f"vab{m}")
               for m in range(N // P)]

        # ---------------- PE warmup (p-state ramp) while DMAs land
        _lbl("warmup")
        nc.gpsimd.memset(warm, 0.0)
        wps = ps.tile([P, SPAN], f32, tag="qk", bufs=2, name="wps")
        for _ in range(NWARM):
            nc.tensor.matmul(wps[:64, :64], warm[:, :64], warm[:, :64],
                             start=True, stop=True)

        # ---------------- input DMAs (SP queue), latency-ordered
        # wall columns are host-reordered [m0, m4, v, m1, m5, m2, m6, m3, m7]
        # so the first-needed stationaries + v weights land first.
        _lbl("dma_in")
        for kk in range(4):
            k2 = slice(2 * kk, 2 * kk + 2)
            nc.sync.dma_start(wall[:, k2, :768], wr[:, k2, :768])
            nc.sync.dma_start(xq[0][:, k2, :], xr[:, k2, 0:SPAN])
        nc.sync.dma_start(misc, misc_d)
        for kk in range(4):
            k2 = slice(2 * kk, 2 * kk + 2)
            nc.sync.dma_start(wall[:, k2, 768:], wr[:, k2, 768:])
            nc.sync.dma_start(xq[1][:, k2, :], xr[:, k2, SPAN:2 * SPAN])
        nc.sync.dma_start(xq[2], xr[:, :, 2 * SPAN:3 * SPAN])
        nc.sync.dma_start(xq[3], xr[:, :, 3 * SPAN:4 * SPAN])
        nc.sync.dma_start(wpk, pr)

        # ---------------- work-unit generators
        # PSUM bank budget (8 banks of [128, 2KB]):
        #   duo  [128, 2, 512]f32 x2   = 4 banks (score pipeline)
        #   qk   [128, 512]f32    x2   = 2 banks (qkv/v/proj filler units)
        #   pvt  [128, 4, 128]f32 x2   = 2 banks (PV accumulators, 1/head)
        WCOL = {0: 0, 4: 128, 1: 768, 5: 896, 2: 1024, 6: 1152,
                3: 1280, 7: 1408}

        def qk_unit(m, q, tag="qk", tbufs=2):
            _lbl(f"qk_unit")
            pg = ps.tile([P, SPAN], f32, tag=tag, bufs=tbufs,
                         name=f"pg{m}{q}")
            for k in range(KC):
                _lbl("qk_unit")
                nc.tensor.matmul(pg, wall[:, k, WCOL[m]:WCOL[m] + P],
                                 xq[q][:, k, :],
                                 start=(k == 0), stop=(k == KC - 1))
                yield SPAN
            nc.vector.tensor_copy(out=qkm[m][q], in_=pg)
            yield 0

        def v_unit(b16, tag="qk", tbufs=2):
            _lbl(f"v_unit")
            nc.gpsimd.memset(vab[b16], 1.0)
            pv = ps.tile([P, SPAN], f32, tag=tag, bufs=tbufs,
                         name=f"pv{b16}")
            qb, cb = divmod(b16, 4)
            for k in range(KC):
                _lbl("v_unit")
                nc.tensor.matmul(pv, xq[qb][:, k, cb * P:(cb + 1) * P],
                                 wall[:, k, 256:768],
                                 start=(k == 0), stop=(k == KC - 1))
                yield SPAN
            nc.vector.tensor_copy(
                out=vab[b16].rearrange("p (h e) -> p h e", h=HPC)[:, :, :D],
                in_=pv.rearrange("p (h e) -> p h e", h=HPC),
            )
            yield 0

        obslot = [None]

        def proj_unit(J, mo, acTc, tag="qk", tbufs=2):
            _lbl(f"proj{J}")
            pp = ps.tile([P, SPAN], f32, tag=tag, name=f"pp{J}{mo}",
                         bufs=tbufs)
            for k in range(GD // P):
                _lbl(f"proj{J}")
                nc.tensor.matmul(pp, wpk[:, k, mo * P:(mo + 1) * P],
                                 acTc[k],
                                 start=(k == 0), stop=(k == GD // P - 1))
                yield SPAN
            if mo % 2 == 0:
                obslot[0] = sp.tile([P, 2, SPAN], bf, tag="ob", bufs=4,
                                    name="ob")
            nc.vector.tensor_copy(out=obslot[0][:, mo % 2, :], in_=pp)
            if J == NSPAN - 1:
                # tail: per-mo output DMAs, alternating between the SP and
                # the (idle-by-now) Act queues so a semaphore wait on one
                # sequencer doesn't delay the next DMA's issue
                eng = nc.sync if mo % 2 == 0 else nc.scalar
                eng.dma_start(
                    outr[:, mo:mo + 1, J * SPAN:(J + 1) * SPAN],
                    obslot[0][:, mo % 2:mo % 2 + 1, :])
            elif mo % 2 == 1:
                nc.sync.dma_start(
                    outr[:, mo - 1:mo + 1, J * SPAN:(J + 1) * SPAN],
                    obslot[0])
            yield 0

        # ---------------- attention
        def attn_pair(J, hp, asmP, filler, sub_done=None):
            nblk = 4 * (J + 1)
            pvt = [ps.tile([P, 4, P], f32, tag="pvt", bufs=2,
                           name=f"pvt{J}{hp}{hi}")
                   for hi in (0, 1)]
            # start_tensor_calc zeroes the whole 2KB PSUM bank, so the four
            # per-subtile accumulation groups sharing one bank must NOT each
            # use start=True.  One dummy F=1 matmul opens + zeroes the bank;
            # every PV matmul then accumulates (start=False), and the final
            # write (s=3 diagonal) closes the group.
            for hi in (0, 1):
                nc.tensor.matmul(pvt[hi][:, 0, P - 1:P], i128, warm[:, 0:1],
                                 start=True, stop=False)

            def emit_pv(ex, j2, dtg):
                _lbl(f"pv_J{J}")
                for s in range(4):
                    if s < dtg:
                        continue
                    for hi in (0, 1):
                        h = 2 * hp + hi
                        nc.tensor.matmul(
                            pvt[hi][:, s, :DP1],
                            ex[:, hi, s * P:(s + 1) * P],
                            vab[j2][:, h * DP1:(h + 1) * DP1],
                            start=False,
                            stop=(s == 3 and j2 == 4 * J + 3),
                        )

            pend = deque()
            for j2 in range(nblk):
                _lbl(f"scores_J{J}")
                duo = ps.tile([P, 2, SPAN], f32, tag="duo", bufs=2, name="duo")
                dtg = j2 - 4 * J
                diag = dtg >= 0
                lo = P * dtg if diag else 0
                for hi in (0, 1):
                    kst = qkm[4 + hp][j2 // 4][64 * hi:64 * (hi + 1),
                                               (j2 % 4) * P:(j2 % 4 + 1) * P]
                    qmv = qkm[hp][J][64 * hi:64 * (hi + 1), :]
                    nc.tensor.matmul(duo[:, hi, lo:], kst, qmv[:, lo:],
                                     start=True, stop=True)
                _lbl(f"exp_J{J}")
                ex = exp_pool.tile([P, 2, SPAN], bf, tag="ex", bufs=8, name="ex")
                nc.scalar.activation(ex[:, :, lo:], duo[:, :, lo:], Exp)
                if diag:
                    # zero the dead (q < kv) triangle of the diagonal block
                    nc.vector.tensor_tensor(ex[:, :, lo:lo + P],
                                            ex[:, :, lo:lo + P], tri2, MUL)
                pend.append((ex, j2, dtg))
                if len(pend) > 3:
                    emit_pv(*pend.popleft())
                filler.pull(QUOTA_J[J])
            while pend:
                emit_pv(*pend.popleft())
            _lbl(f"norm_J{J}")
            rcb = [sp.tile([P, 4, 1], f32, tag=f"rc{hi}", bufs=2,
                           name=f"rcb{hi}") for hi in (0, 1)]
            for hi in (0, 1):
                nc.vector.reciprocal(rcb[hi], pvt[hi][:, :, D:DP1])
            for s in range(4):
                for hi in (0, 1):
                    nc.vector.tensor_scalar(
                        asmP[:, s, hi * D:(hi + 1) * D],
                        pvt[hi][:, s, :D], rcb[hi][:, s, :], None, MUL)
                if sub_done is not None:
                    sub_done(s)

        # ---------------- schedule
        def drain_rr(gens):
            gens = list(gens)
            while gens:
                gens = [g for g in gens if next(g, None) is not None]

        # Phase A: first qk chunks + V blocks 0..3 (paced by input DMAs).
        # Borrow the idle duo ring so three units can be in flight at once.
        drain_rr([qk_unit(0, 0), qk_unit(4, 0, "duo", 2), v_unit(0, "duo", 2)])
        drain_rr([v_unit(1), v_unit(2, "duo", 2), v_unit(3, "duo", 2)])

        filler = _Filler()
        order = []
        for J in range(4):
            for pair in (1, 2, 3):
                order += [(f"qk{pair}_{J}", qk_unit(pair, J)),
                          (f"qk{4 + pair}_{J}", qk_unit(4 + pair, J))]
            if J < 3:
                order += [(f"qk0_{J + 1}", qk_unit(0, J + 1)),
                          (f"qk4_{J + 1}", qk_unit(4, J + 1))]
                order += [(f"v{b}", v_unit(b))
                          for b in range(4 * (J + 1), 4 * (J + 1) + 4)]
        for name, gen in order:
            filler.add(name, gen)

        for J in range(NSPAN):
            acTc = [aTp.tile([P, SPAN], bf, tag=f"acT{k}", bufs=3,
                             name=f"acT{J}_{k}") for k in range(GD // P)]
            for hp in range(4):
                need = [f"qk{hp}_{J}", f"qk{4 + hp}_{J}"]
                if hp == 0 and J > 0:
                    need += [f"v{b}" for b in range(4 * J, 4 * J + 4)]
                filler.ensure([n for n in need
                               if any(n == nm for nm, _ in order)])
                asmP = aTp.tile([P, 4, P], bf, tag=f"asmp{hp}", bufs=2,
                                name=f"asmp{J}_{hp}")
                if J == NSPAN - 1 and hp == 3:
                    # The last pair gates the tail: PE transpose + DVE
                    # copy-out per completed subtile beats the DMA-XBAR
                    # round trip.  Transposes alternate between the duo and
                    # the (now idle) pvt rings so each lands in its own
                    # PSUM zero region with no buffer-turnaround stalls.
                    def sub_done(s, _hp=hp):
                        _lbl(f"transpose_J{NSPAN - 1}")
                        tps = ps.tile([P, P], bf,
                                      tag="duo" if s < 2 else "pvt",
                                      bufs=2, name=f"tps{s}")
                        nc.tensor.transpose(tps, asmP[:, s, :], i128)
                        nc.vector.tensor_copy(
                            out=acTc[_hp][:, s * P:(s + 1) * P], in_=tps)

                    attn_pair(J, hp, asmP, filler, sub_done)
                else:
                    attn_pair(J, hp, asmP, filler)
                    _lbl(f"transpose_J{J}")
                    nc.sync.dma_start(
                        acTc[hp].rearrange("p (s q) -> p s q", s=4),
                        asmP, transpose=True)
            for mo in range(C // P):
                filler.add(f"proj{J}_{mo}", proj_unit(J, mo, acTc))
        filler.run_all()


def _emit(tc, mybir, reps=1, phases=None):
    nc = tc.nc
    dt = mybir.dt
    f32, bf = dt.float32, dt.bfloat16

    xT_d = nc.dram_tensor("xT", [C, N], bf, kind="ExternalInput").ap()
    wall_d = nc.dram_tensor("WALL", [C, 3 * GD], bf,
                            kind="ExternalInput").ap()
    wpT_d = nc.dram_tensor("wpT", [GD, C], bf, kind="ExternalInput").ap()
    misc_d = nc.dram_tensor("MISC", [P, 3 * P], bf,
                            kind="ExternalInput").ap()
    out_d = nc.dram_tensor("outT", [C, N], bf, kind="ExternalOutput").ap()

    for _rep in range(reps):
        _emit_once(tc, mybir, xT_d, wall_d, wpT_d, misc_d, out_d)


def _get_module(reps=1, phases=None):
    key = (reps,)
    if key not in _CACHE:
        import concourse.tile as tile
        from concourse import bacc, mybir

        nc = bacc.Bacc("TRN2", target_bir_lowering=False, debug=False,
                       num_devices=8)
        with tile.TileContext(nc) as tc:
            _emit(tc, mybir, reps=reps)
        nc.compile()
        _CACHE[key] = nc
    return _CACHE[key]


def _host_inputs(x, w_qkv, w_proj):
    scale = D ** -0.5
    tri01 = np.zeros((P, P), np.float32)
    for p in range(P):
        tri01[p, p:] = 1.0
    misc = np.concatenate([np.eye(P, dtype=np.float32), tri01, tri01],
                          axis=1).astype(BF16)
    in_maps = []
    for core in range(8):
        b, g = core // 2, core % 2
        rows = slice(g * GD, (g + 1) * GD)
        wq = (w_qkv[0 * C:1 * C][rows] * scale).T
        wk = w_qkv[1 * C:2 * C][rows].T
        wv = w_qkv[2 * C:3 * C][rows].T
        # column order [m0, m4, v, m1, m5, m2, m6, m3, m7] (see WCOL)
        wall = np.concatenate(
            [wq[:, 0:128], wk[:, 0:128], wv,
             wq[:, 128:256], wk[:, 128:256],
             wq[:, 256:384], wk[:, 256:384],
             wq[:, 384:512], wk[:, 384:512]], axis=1)  # [C, 1536]
        in_maps.append({
            "xT": np.ascontiguousarray(x[b].T).astype(BF16),
            "WALL": np.ascontiguousarray(wall).astype(BF16),
            "wpT": np.ascontiguousarray(w_proj[:, rows].T).astype(BF16),
            "MISC": misc,
        })
    return in_maps


def kernel(x, w_qkv, w_proj, b_proj, _trace=False):
    from concourse.bass_utils import run_bass_kernel_spmd

    nc = _get_module()
    in_maps = _host_inputs(np.asarray(x, np.float32),
                           np.asarray(w_qkv, np.float32),
                           np.asarray(w_proj, np.float32))
    res = run_bass_kernel_spmd(nc, in_maps, core_ids=list(range(8)),
                               trace=_trace)
    outs = [np.asarray(r["outT"], np.float32) for r in res.results]
    out = np.empty((B, N, C), np.float32)
    bp = np.asarray(b_proj, np.float32)[None, :]
    for b in range(B):
        out[b] = outs[2 * b].T + outs[2 * b + 1].T + bp
    if _trace:
        kernel._last_results = res
    return out
